# revision 1
# baseline (speedup 1.0000x reference)
"""Trainium2 Bass kernel for nn_CrossNetwork: 4-layer cross-network.

Reference semantics (per row b of x [B, D], D=512, L=4 layers):
    x_list = [x]
    for i in range(L):
        h = x_list[-1]
        for p in x_list[:-1]:          # sequential dot-product residuals
            s = <h_cur, p>             # scalar per row (h_cur updated each step)
            h_cur = h_cur + s * ones
        y = h_cur @ W[i].T + b[i]
        x_list.append(y)
    out = concat(x_list[1:])           # [B, L*D]

Key algebraic restructure (exact): adding a per-row scalar s to every
component only shifts later dot products by s * rowsum(prior).  With
D_j = <h, p_j> (h = the layer input, unmodified) and sig_j = rowsum(p_j):
    s'_j = D_j + S_{<j} * sig_j ;  S = sum_j s'_j
so only the plain dots D_j, the row-sums sig_j of y0/y1, and a tiny
per-row recurrence are needed; the shift S is applied once per layer.

Layout: batch rows on SBUF partitions ([128, 512] tiles), activations f32.
Matmul stationary = PE-transposed activation chunks; moving = host-
pre-transposed W^T.  Bias via an extra K=1 accumulating matmul.
Sharding: batch split across 8 NeuronCores (data parallel, SPMD).
"""

import numpy as np

NUM_LAYERS = 4
D = 512
B = 16384
N_CORES = 8
ROWS_PER_CORE = B // N_CORES          # 2048
NTILES = ROWS_PER_CORE // 128         # 16
NCH = D // 128                        # 4 contraction chunks

# matmul operand dtype: "bf16" or "f32r"
MM_DTYPE = "f32r"
# row-dot reduction: "ts_accum" (mul + tensor_scalar reduce) or
# "reduce" (mul + tensor_reduce)
DOT_MODE = "ts_accum"

_CACHE = {}


def _build_nc(ntiles=NTILES):
    import concourse.tile as tile
    from concourse import bacc, mybir
    from concourse.masks import make_identity

    F32 = mybir.dt.float32
    BF16 = mybir.dt.bfloat16
    F32R = mybir.dt.float32r
    AF = mybir.ActivationFunctionType
    MUL = mybir.AluOpType.mult
    ADD = mybir.AluOpType.add

    MMDT = F32R if MM_DTYPE == "f32r" else BF16
    FINDT = F32 if MM_DTYPE == "f32r" else BF16
    rows = ntiles * 128

    nc = bacc.Bacc("TRN2", target_bir_lowering=False, debug=False)

    X = nc.dram_tensor("x", [rows, D], F32, kind="ExternalInput")
    WT = nc.dram_tensor("wt", [NUM_LAYERS, D, D], MMDT, kind="ExternalInput")
    BIAS = nc.dram_tensor("bias", [NUM_LAYERS, D], MMDT, kind="ExternalInput")
    OUT = nc.dram_tensor("out", [rows, NUM_LAYERS * D], F32,
                         kind="ExternalOutput")

    with tile.TileContext(nc) as tc:
        with (
            tc.tile_pool(name="consts", bufs=1) as consts,
            tc.tile_pool(name="acts", bufs=2) as acts,
            tc.tile_pool(name="fins", bufs=3) as fins,
            tc.tile_pool(name="scratch", bufs=2) as scratch,
            tc.tile_pool(name="scals", bufs=2) as scals,
            tc.tile_pool(name="ypsum", bufs=3, space="PSUM") as ypsum,
            tc.tile_pool(name="tpsum", bufs=3, space="PSUM") as tpsum,
        ):
            # ---- constants (loaded once) ----
            wt_sb = consts.tile([128, NUM_LAYERS, NCH, D], MMDT)
            wt_dram = WT.rearrange("l (c p) e -> l c p e", p=128)
            for i in range(NUM_LAYERS):
                for c in range(NCH):
                    nc.sync.dma_start(wt_sb[:, i, c, :], wt_dram[i, c, :, :])
            bias_sb = consts.tile([1, NUM_LAYERS, D], MMDT)
            for i in range(NUM_LAYERS):
                nc.sync.dma_start(bias_sb[0:1, i, :], BIAS[i:i + 1, :])
            ones_f32 = consts.tile([1, 128], F32)
            nc.vector.memset(ones_f32[:], 1.0)
            ones_row = consts.tile([1, 128], MMDT)
            nc.vector.tensor_copy(ones_row[:], ones_f32[:])
            ident = consts.tile([128, 128], FINDT)
            make_identity(nc, ident[:])

            x_dram = X.rearrange("(t p) d -> t p d", p=128)
            out_dram = OUT.rearrange("(t p) d -> t p d", p=128)

            def row_reduce(src_ap, dst_col, tag):
                """dst_col[128,1] = rowsum(src_ap [128,D])."""
                if DOT_MODE == "ts_accum":
                    waste = scratch.tile([128, D], F32, tag=tag)
                    nc.vector.tensor_scalar(
                        out=waste[:], in0=src_ap, scalar1=0.0, scalar2=None,
                        op0=ADD, op1=ADD, accum_out=dst_col)
                else:
                    nc.vector.tensor_reduce(
                        out=dst_col, in_=src_ap, op=ADD,
                        axis=mybir.AxisListType.X)

            for t in range(ntiles):
                # ---- load x tile ----
                x_t = acts.tile([128, D], F32, tag="x")
                nc.sync.dma_start(x_t[:], x_dram[t, :, :])

                ys = []      # f32 activation tiles [x_t, y0, y1, y2]
                sigs = {}    # rowsum columns for y0, y1

                scal = scals.tile([128, 16], F32, tag="scal")
                ncol = [0]
                def col():
                    c = ncol[0]; ncol[0] += 1
                    return scal[:, c:c + 1]

                h = x_t
                ys.append(x_t)

                for i in range(NUM_LAYERS):
                    # ---- dots vs priors + recurrence -> S (skip layer 0) ----
                    S = None
                    if i >= 1:
                        Ds = []
                        for j, p in enumerate(ys[:-1]):
                            prod = scratch.tile([128, D], F32, tag="prod")
                            nc.vector.tensor_tensor(
                                out=prod[:], in0=h[:], in1=p[:], op=MUL)
                            Dj = col()
                            row_reduce(prod[:], Dj, "dotred")
                            Ds.append(Dj)
                        if i == 1:
                            S = Ds[0]
                        elif i == 2:
                            # S = D0 + D1 + D0*sig(y0)
                            u = col()
                            nc.vector.tensor_scalar(
                                out=u, in0=sigs[0], scalar1=Ds[0], scalar2=Ds[0],
                                op0=MUL, op1=ADD)  # u = sig0*D0 + D0
                            S = col()
                            nc.vector.tensor_scalar(
                                out=S, in0=u, scalar1=Ds[1], scalar2=None, op0=ADD)
                        else:
                            # priors x, y0, y1 with sig(y0), sig(y1)
                            u = col()
                            nc.vector.tensor_scalar(
                                out=u, in0=sigs[0], scalar1=Ds[0], scalar2=Ds[0],
                                op0=MUL, op1=ADD)          # u = D0*(1+sig0)
                            sa = col()
                            nc.vector.tensor_scalar(
                                out=sa, in0=u, scalar1=Ds[1], scalar2=None, op0=ADD)
                            v = col()
                            nc.vector.tensor_scalar(
                                out=v, in0=sigs[1], scalar1=sa, scalar2=sa,
                                op0=MUL, op1=ADD)          # v = sa*(1+sig1)
                            S = col()
                            nc.vector.tensor_scalar(
                                out=S, in0=v, scalar1=Ds[2], scalar2=None, op0=ADD)

                    # ---- x_fin = h + S (gpsimd; casts when FINDT != F32) ----
                    if S is None:
                        if FINDT == F32:
                            x_fin = h
                        else:
                            x_fin = fins.tile([128, D], FINDT, tag="fin")
                            nc.gpsimd.tensor_copy(x_fin[:], h[:])
                    else:
                        x_fin = fins.tile([128, D], FINDT, tag="fin")
                        nc.gpsimd.tensor_scalar_add(x_fin[:], h[:], S)

                    # ---- transpose x_fin -> stationary chunks ----
                    tr = tpsum.tile([128, NCH, 128], FINDT, tag="tr")
                    for c in range(NCH):
                        nc.tensor.transpose(
                            tr[:, c, :], x_fin[:, c * 128:(c + 1) * 128], ident[:])
                    xT = fins.tile([128, NCH, 128], MMDT, tag="xT")
                    nc.scalar.activation(xT[:], tr[:], AF.Copy)

                    # ---- matmuls: y = x_fin @ W_i^T + bias ----
                    y_ps = ypsum.tile([128, D], F32, tag="y")
                    for c in range(NCH):
                        nc.tensor.matmul(
                            y_ps[:], xT[:, c, :], wt_sb[:, i, c, :],
                            start=(c == 0), stop=False)
                    nc.tensor.matmul(
                        y_ps[:], ones_row[:], bias_sb[:, i, :],
                        start=False, stop=True)

                    # ---- P1: copy y psum -> sbuf f32; sigma for y0, y1 ----
                    y = acts.tile([128, D], F32, tag=f"y{i}")
                    nc.scalar.activation(y[:], y_ps[:], AF.Copy)
                    if i in (0, 1):
                        sig = col()
                        row_reduce(y[:], sig, "sigred")
                        sigs[i] = sig

                    # ---- DMA out ----
                    nc.sync.dma_start(out_dram[t, :, i * D:(i + 1) * D], y[:])

                    ys.append(y)
                    h = y

    nc.compile()
    return nc


def _host_prep(W, b):
    """W [L,D,D] f32 (torch Linear layout: y = x @ W.T) -> transposed WT[l,d,e]."""
    WT = np.ascontiguousarray(W.transpose(0, 2, 1))
    bias = np.ascontiguousarray(b)
    if MM_DTYPE == "f32r":
        # PE accepts raw f32 bits for f32r DRAM operands (verified on HW:
        # identical error to DVE-rounded) — no host rounding needed.
        return WT, bias
    else:
        import ml_dtypes
        return (np.asarray(WT, dtype=ml_dtypes.bfloat16),
                np.asarray(bias, dtype=ml_dtypes.bfloat16))


def run_shards(x, W, b, **spmd_kwargs):
    """Run the SPMD kernel; returns (full_output, BassKernelResults)."""
    from concourse.bass_utils import run_bass_kernel_spmd

    x = np.ascontiguousarray(np.asarray(x, np.float32))
    WT, bias = _host_prep(np.asarray(W, np.float32), np.asarray(b, np.float32))

    if "nc" not in _CACHE:
        _CACHE["nc"] = _build_nc()
    nc = _CACHE["nc"]

    in_maps = []
    for c in range(N_CORES):
        shard = x[c * ROWS_PER_CORE:(c + 1) * ROWS_PER_CORE]
        in_maps.append({"x": np.ascontiguousarray(shard), "wt": WT, "bias": bias})

    res = run_bass_kernel_spmd(nc, in_maps, core_ids=list(range(N_CORES)),
                               **spmd_kwargs)
    out = np.concatenate([r["out"] for r in res.results], axis=0)
    return out.astype(np.float32), res


def kernel(x, W, b):
    out, _ = run_shards(x, W, b)
    return out



# revision 3
# speedup vs baseline: 1.8718x; 1.8718x over previous
"""Trainium2 Bass kernel for nn_CrossNetwork: 4-layer cross-network.

Reference semantics (per row b of x [B, D], D=512, L=4 layers):
    x_list = [x]
    for i in range(L):
        h = x_list[-1]
        for p in x_list[:-1]:          # sequential dot-product residuals
            s = <h_cur, p>             # scalar per row (h_cur updated each step)
            h_cur = h_cur + s * ones
        y = h_cur @ W[i].T + b[i]
        x_list.append(y)
    out = concat(x_list[1:])           # [B, L*D]

Key algebraic restructure (exact): adding a per-row scalar s to every
component only shifts later dot products by s * rowsum(prior).  With
D_j = <h, p_j> (h = the layer input, unmodified) and sig_j = rowsum(p_j):
    s'_j = D_j + S_{<j} * sig_j ;  S = sum_j s'_j
so only the plain dots D_j, the row-sums sig_j of y0/y1, and a tiny
per-row recurrence are needed; the shift S is applied once per layer.

v2 design (vs v1 baseline at 855us HW):
- bf16 everywhere (weights, activations, output DRAM); PSUM stays f32.
  Output upcast to f32 on host.
- Per-layer activation transpose done by the DMA xbar (one
  dma_start_transpose per layer-tile producing [r, c, p] chunk layout)
  instead of 4 PE transposes + ACT cast: PE runs only real matmuls.
- x_fin = h + S on DVE tensor_scalar (bf16 4x mode) instead of GpSimd
  (which measured 7.5us per op = 360us total in v1).
- Dot products fused: one tensor_tensor_reduce per dot (mult + row
  reduce in a single DVE op).
- sig row-sums fused into the ACT PSUM->SBUF evacuation via accum_out.
- One output DMA per tile ([128, 4, 512] all layers at once).

Sharding: batch split across 8 NeuronCores (data parallel, SPMD).
"""

import numpy as np

NUM_LAYERS = 4
D = 512
B = 16384
N_CORES = 8
ROWS_PER_CORE = B // N_CORES          # 2048
NTILES = ROWS_PER_CORE // 128         # 16
NCH = D // 128                        # 4 contraction chunks

_CACHE = {}


def _build_nc(ntiles=NTILES):
    import concourse.tile as tile
    from concourse import bacc, mybir

    F32 = mybir.dt.float32
    BF16 = mybir.dt.bfloat16
    AF = mybir.ActivationFunctionType
    MUL = mybir.AluOpType.mult
    ADD = mybir.AluOpType.add

    rows = ntiles * 128

    nc = bacc.Bacc("TRN2", target_bir_lowering=False, debug=False)

    X = nc.dram_tensor("x", [rows, D], BF16, kind="ExternalInput")
    WT = nc.dram_tensor("wt", [NUM_LAYERS, D, D], BF16, kind="ExternalInput")
    BIAS = nc.dram_tensor("bias", [NUM_LAYERS, D], BF16, kind="ExternalInput")
    OUT = nc.dram_tensor("out", [rows, NUM_LAYERS, D], BF16,
                         kind="ExternalOutput")

    with tile.TileContext(nc) as tc:
        with (
            tc.tile_pool(name="consts", bufs=1) as consts,
            tc.tile_pool(name="acts", bufs=3) as acts,
            tc.tile_pool(name="fins", bufs=3) as fins,
            tc.tile_pool(name="scratch", bufs=2) as scratch,
            tc.tile_pool(name="scals", bufs=3) as scals,
            tc.tile_pool(name="ypsum", bufs=4, space="PSUM") as ypsum,
        ):
            # ---- constants (loaded once) ----
            wt_sb = consts.tile([128, NUM_LAYERS, NCH, D], BF16)
            wt_dram = WT.rearrange("l (c p) e -> l c p e", p=128)
            for i in range(NUM_LAYERS):
                for c in range(NCH):
                    nc.sync.dma_start(wt_sb[:, i, c, :], wt_dram[i, c, :, :])
            bias_sb = consts.tile([1, NUM_LAYERS, D], BF16)
            for i in range(NUM_LAYERS):
                nc.sync.dma_start(bias_sb[0:1, i, :], BIAS[i:i + 1, :])
            ones_f32 = consts.tile([1, 128], F32)
            nc.vector.memset(ones_f32[:], 1.0)
            ones_row = consts.tile([1, 128], BF16)
            nc.vector.tensor_copy(ones_row[:], ones_f32[:])

            x_dram = X.rearrange("(t p) d -> t p d", p=128)
            out_dram = OUT.rearrange("(t p) l d -> t p l d", p=128)

            for t in range(ntiles):
                # all activations for this tile: [x, y0, y1, y2, y3]
                ysb = acts.tile([128, NUM_LAYERS + 1, D], BF16, tag="acts")
                nc.sync.dma_start(ysb[:, 0, :], x_dram[t, :, :])
                ys = [ysb[:, j, :] for j in range(NUM_LAYERS + 1)]

                scal = scals.tile([128, 16], F32, tag="scal")
                ncol = [0]
                def col():
                    c = ncol[0]; ncol[0] += 1
                    return scal[:, c:c + 1]

                sigs = {}

                for i in range(NUM_LAYERS):
                    h = ys[i]
                    # ---- dots vs priors + recurrence -> S (skip layer 0) ----
                    S = None
                    if i >= 1:
                        Ds = []
                        for j in range(i):
                            waste = scratch.tile([128, D], BF16, tag="prod")
                            Dj = col()
                            nc.vector.scalar_tensor_tensor(
                                out=waste[:], in0=h, scalar=1.0, in1=ys[j],
                                op0=MUL, op1=MUL, accum_out=Dj)
                            Ds.append(Dj)
                        if i == 1:
                            S = Ds[0]
                        elif i == 2:
                            # S = D0 + D1 + D0*sig(y0)
                            u = col()
                            nc.vector.tensor_scalar(
                                out=u, in0=sigs[0], scalar1=Ds[0], scalar2=Ds[0],
                                op0=MUL, op1=ADD)  # u = sig0*D0 + D0
                            S = col()
                            nc.vector.tensor_scalar(
                                out=S, in0=u, scalar1=Ds[1], scalar2=None, op0=ADD)
                        else:
                            # priors x, y0, y1 with sig(y0), sig(y1)
                            u = col()
                            nc.vector.tensor_scalar(
                                out=u, in0=sigs[0], scalar1=Ds[0], scalar2=Ds[0],
                                op0=MUL, op1=ADD)          # u = D0*(1+sig0)
                            sa = col()
                            nc.vector.tensor_scalar(
                                out=sa, in0=u, scalar1=Ds[1], scalar2=None, op0=ADD)
                            v = col()
                            nc.vector.tensor_scalar(
                                out=v, in0=sigs[1], scalar1=sa, scalar2=sa,
                                op0=MUL, op1=ADD)          # v = sa*(1+sig1)
                            S = col()
                            nc.vector.tensor_scalar(
                                out=S, in0=v, scalar1=Ds[2], scalar2=None, op0=ADD)

                    # ---- x_fin = h + S on DVE (bf16 4x mode) ----
                    if S is None:
                        x_fin_ap = h
                    else:
                        x_fin = fins.tile([128, D], BF16, tag="fin")
                        nc.vector.tensor_scalar(
                            out=x_fin[:], in0=h, scalar1=S, scalar2=None,
                            op0=ADD)
                        x_fin_ap = x_fin[:]

                    # ---- transpose via DMA xbar: xT[r, c, p] = x_fin[p, c*128+r]
                    xT = fins.tile([128, NCH, 128], BF16, tag="xT")
                    nc.sync.dma_start_transpose(xT[:], x_fin_ap)

                    # ---- matmuls: y = x_fin @ W_i^T + bias ----
                    y_ps = ypsum.tile([128, D], F32, tag="y")
                    for c in range(NCH):
                        nc.tensor.matmul(
                            y_ps[:], xT[:, c, :], wt_sb[:, i, c, :],
                            start=(c == 0), stop=False)
                    nc.tensor.matmul(
                        y_ps[:], ones_row[:], bias_sb[:, i, :],
                        start=False, stop=True)

                    # ---- evac PSUM -> SBUF bf16 on ACT; fused sig rowsum ----
                    if i in (0, 1):
                        sig = col()
                        nc.scalar.activation(ys[i + 1], y_ps[:], AF.Copy,
                                             accum_out=sig)
                        sigs[i] = sig
                    else:
                        nc.scalar.activation(ys[i + 1], y_ps[:], AF.Copy)

                # ---- one DMA out per tile: all 4 layer outputs ----
                nc.sync.dma_start(out_dram[t, :, :, :], ysb[:, 1:, :])

    nc.compile()
    return nc


def _host_prep(x, W, b):
    """Cast inputs to bf16; W [L,D,D] (torch Linear layout: y = x @ W.T)
    -> transposed WT[l,d,e]."""
    import ml_dtypes
    BF = ml_dtypes.bfloat16
    xb = np.asarray(x, np.float32).astype(BF)
    WTb = np.ascontiguousarray(
        np.asarray(W, np.float32).transpose(0, 2, 1)).astype(BF)
    bb = np.asarray(b, np.float32).astype(BF)
    return xb, WTb, bb


def run_shards(x, W, b, **spmd_kwargs):
    """Run the SPMD kernel; returns (full_output, BassKernelResults)."""
    from concourse.bass_utils import run_bass_kernel_spmd

    xb, WTb, bb = _host_prep(x, W, b)

    if "nc" not in _CACHE:
        _CACHE["nc"] = _build_nc()
    nc = _CACHE["nc"]

    in_maps = []
    for c in range(N_CORES):
        shard = xb[c * ROWS_PER_CORE:(c + 1) * ROWS_PER_CORE]
        in_maps.append({"x": np.ascontiguousarray(shard), "wt": WTb,
                        "bias": bb})

    res = run_bass_kernel_spmd(nc, in_maps, core_ids=list(range(N_CORES)),
                               **spmd_kwargs)
    # out: [rows, L, D] bf16 per core -> [B, L*D] f32
    out = np.concatenate(
        [r["out"].reshape(ROWS_PER_CORE, NUM_LAYERS * D) for r in res.results],
        axis=0)
    return out.astype(np.float32), res


def kernel(x, W, b):
    out, _ = run_shards(x, W, b)
    return out


# revision 12
# speedup vs baseline: 5.5367x; 2.9580x over previous
"""Trainium2 Bass kernel for nn_CrossNetwork: 4-layer cross-network.

Reference semantics (per row b of x [B, D], D=512, L=4 layers):
    x_list = [x]
    for i in range(L):
        h = x_list[-1]
        for p in x_list[:-1]:          # sequential dot-product residuals
            s = <h_cur, p>             # scalar per row (h_cur updated each step)
            h_cur = h_cur + s * ones
        y = h_cur @ W[i].T + b[i]
        x_list.append(y)
    out = concat(x_list[1:])           # [B, L*D]

Algebraic restructure (exact): with D_j = <h, p_j> (h unmodified) and
sig_j = rowsum(p_j), the accumulated shift is
    S_2 = D_0;  S_3 = D_0(1+sig_0) + D_1;
    S_4 = (D_0(1+sig_0) + D_1)(1+sig_1) + D_2
and y_i = h W_i^T + S_i * wsum_i + b_i  (wsum_i = rowsum of W_i), i.e.
the shift never needs to be materialized into the activation.

v4 design — fully transposed activations (vs v3 row-major at 457us):
- Activations live as y^T chunks [128(d), NCH, NB(b)] in SBUF.  The PE
  consumes them directly as the MOVING operand with W^T chunks as
  stationary: y^T[e,b] = sum_d W^T[d,e] h^T[d,b].  NO transposes exist
  anywhere in the kernel (v3 spent 80us of Sync-engine time on DMA
  xbar transposes; v1 spent ~130us of PE time on PE transposes).
- Host pre-transposes x (free) and post-transposes the output (free);
  DRAM tensors are already in transposed layout.
- Per-row dot products: DVE computes the elementwise product (one
  bf16 tensor_tensor over all 4 chunks), PE reduces across partitions
  with a ones-column stationary matmul accumulating the 4 chunks into
  a [1, NB] PSUM row.  Dots land as ROWS, which is exactly the form
  the aux matmul needs - no column->row conversion.
- bias + shift applied in one K=2 aux matmul per e-chunk: stationary
  [bias_i; wsum_i], moving [ones; S^T].
- recurrence for S on [1, NB] rows via DVE tensor_tensor.
- PSUM -> SBUF bf16 evacuation on the Scalar engine.

Sharding: batch split across 8 NeuronCores (data parallel, SPMD).
"""

import numpy as np

NUM_LAYERS = 4
D = 512
B = 16384
N_CORES = 8
ROWS_PER_CORE = B // N_CORES          # 2048
NB = 512                              # batch-columns per tile
NBT = ROWS_PER_CORE // NB             # 4 b-tiles
NCH = D // 128                        # 4 contraction chunks

_CACHE = {}


def _build_nc(nbt=NBT):
    import concourse.tile as tile
    from concourse import bacc, mybir

    F32 = mybir.dt.float32
    BF16 = mybir.dt.bfloat16
    AF = mybir.ActivationFunctionType
    MUL = mybir.AluOpType.mult
    ADD = mybir.AluOpType.add

    rows = nbt * NB

    nc = bacc.Bacc("TRN2", target_bir_lowering=False, debug=False)

    # x^T: xt[c, p, b] = x[b, c*128+p]
    XT = nc.dram_tensor("xt", [NCH, 128, rows], BF16, kind="ExternalInput")
    # wt[l, d, e] = W[l, e, d]
    WT = nc.dram_tensor("wt", [NUM_LAYERS, D, D], BF16, kind="ExternalInput")
    # aux[0, l, e] = wsum = rowsum(W_l);  aux[1, l, e] = bias
    AUX = nc.dram_tensor("aux", [2, NUM_LAYERS, D], BF16, kind="ExternalInput")
    # out[l, c, p, b] = y_l[b, c*128+p]
    OUT = nc.dram_tensor("out", [NUM_LAYERS, NCH, 128, rows], BF16,
                         kind="ExternalOutput")

    with tile.TileContext(nc) as tc:
        with (
            tc.tile_pool(name="consts", bufs=1) as consts,
            tc.tile_pool(name="acts", bufs=3) as acts,
            tc.tile_pool(name="scratch", bufs=2) as scratch,
            tc.tile_pool(name="rows", bufs=3) as rowp,
            tc.tile_pool(name="ypsum", bufs=5, space="PSUM") as ypsum,
            tc.tile_pool(name="dotps", bufs=3, space="PSUM") as dotps,
        ):
            # ---- constants (loaded once) ----
            wt_sb = consts.tile([128, NUM_LAYERS, NCH, D], BF16)
            wt_dram = WT.rearrange("l (c p) e -> l c p e", p=128)
            for i in range(NUM_LAYERS):
                for c in range(NCH):
                    nc.sync.dma_start(wt_sb[:, i, c, :], wt_dram[i, c, :, :])
            aux_sb = consts.tile([2, NUM_LAYERS, D], BF16)
            nc.sync.dma_start(aux_sb[:, :, :], AUX[:, :, :])
            # layer-0 bias row, separately so it starts at partition 0
            bias0_sb = consts.tile([1, D], BF16)
            nc.sync.dma_start(bias0_sb[0:1, :], AUX[1:2, 0, :])
            ones_col = consts.tile([128, 1], BF16)
            nc.vector.memset(ones_col[:], 1.0)
            # aux moving slots: partition 0 = S rows (one slot per
            # (layer>=1, b-tile), written by DVE), partition 1 = ones.
            # DVE/ACT cannot write at partition 1, so the ones row is
            # initialized once via SBUF->SBUF DMA from a memset row.
            NSLOT = (NUM_LAYERS - 1) * nbt
            ones_mv = consts.tile([1, max(NB, NSLOT * NB)], BF16)
            nc.vector.memset(ones_mv[:], 1.0)
            perm_mv = consts.tile([2, NSLOT, NB], BF16)
            nc.sync.dma_start(
                perm_mv[1:2, :, :],
                ones_mv[0:1, 0:NSLOT * NB].rearrange("a (s n) -> a s n", n=NB))

            xt_view = XT.rearrange("c p b -> p c b")
            out_view = OUT.rearrange("l c p b -> l p c b")

            for t in range(nbt):
                bsl = slice(t * NB, (t + 1) * NB)

                xt_sb = acts.tile([128, NCH, NB], BF16, tag="xT")
                nc.sync.dma_start(xt_sb[:], xt_view[:, :, bsl])

                ysT = [xt_sb]
                os_rows = {}   # 1 + sig_j rows, SBUF f32

                for i in range(NUM_LAYERS):
                    hT = ysT[-1]

                    # ---- dots vs priors -> PSUM rows; recurrence -> S row --
                    slot_mv = None
                    if i >= 1:
                        Drows = []
                        for j, pT in enumerate(ysT[:-1]):
                            prod = scratch.tile([128, NCH, NB], BF16,
                                                tag="prod")
                            nc.vector.tensor_tensor(
                                out=prod[:], in0=hT[:], in1=pT[:], op=MUL)
                            dps = dotps.tile([1, NB], F32, tag="dot")
                            for c in range(NCH):
                                nc.tensor.matmul(
                                    dps[:], ones_col[:], prod[:, c, :],
                                    start=(c == 0), stop=(c == NCH - 1))
                            Drows.append(dps)

                        slot = (i - 1) * nbt + t
                        slot_mv = perm_mv[:, slot, :]
                        Srow = perm_mv[0:1, slot, :]
                        if i == 1:
                            nc.vector.tensor_copy(Srow, Drows[0][:])
                        elif i == 2:
                            tr = rowp.tile([1, NB], F32, tag="t0")
                            nc.vector.tensor_tensor(
                                out=tr[:], in0=Drows[0][:],
                                in1=os_rows[0][:], op=MUL)
                            nc.vector.tensor_tensor(
                                out=Srow, in0=tr[:], in1=Drows[1][:], op=ADD)
                        else:
                            tr = rowp.tile([1, NB], F32, tag="t0")
                            nc.vector.tensor_tensor(
                                out=tr[:], in0=Drows[0][:],
                                in1=os_rows[0][:], op=MUL)
                            t2 = rowp.tile([1, NB], F32, tag="t1")
                            nc.vector.tensor_tensor(
                                out=t2[:], in0=tr[:], in1=Drows[1][:], op=ADD)
                            t3 = rowp.tile([1, NB], F32, tag="t2")
                            nc.vector.tensor_tensor(
                                out=t3[:], in0=t2[:], in1=os_rows[1][:],
                                op=MUL)
                            nc.vector.tensor_tensor(
                                out=Srow, in0=t3[:], in1=Drows[2][:], op=ADD)

                    # ---- matmuls: y^T = W_i h^T (+ bias + S*wsum) ----
                    yT = acts.tile([128, NCH, NB], BF16, tag=f"y{i}")
                    for ec in range(NCH):
                        esl = slice(ec * 128, (ec + 1) * 128)
                        yps = ypsum.tile([128, NB], F32, tag="yps")
                        for dc in range(NCH):
                            nc.tensor.matmul(
                                yps[:], wt_sb[:, i, dc, esl], hT[:, dc, :],
                                start=(dc == 0), stop=False)
                        if slot_mv is None:
                            nc.tensor.matmul(
                                yps[:], bias0_sb[0:1, esl], ones_mv[0:1, 0:NB],
                                start=False, stop=True)
                        else:
                            nc.tensor.matmul(
                                yps[:], aux_sb[0:2, i, esl], slot_mv,
                                start=False, stop=True)
                        nc.scalar.activation(yT[:, ec, :], yps[:], AF.Copy)

                    # ---- sig rows (1 + rowsum) for y0, y1 ----
                    if i in (0, 1):
                        sps = dotps.tile([1, NB], F32, tag="dot")
                        for c in range(NCH):
                            nc.tensor.matmul(
                                sps[:], ones_col[:], yT[:, c, :],
                                start=(c == 0), stop=(c == NCH - 1))
                        osr = rowp.tile([1, NB], F32, tag="os")
                        nc.vector.tensor_scalar(
                            out=osr[:], in0=sps[:], scalar1=1.0, scalar2=None,
                            op0=ADD)
                        os_rows[i] = osr

                    # ---- DMA out ----
                    nc.sync.dma_start(out_view[i, :, :, bsl], yT[:])

                    ysT.append(yT)

    nc.compile()
    return nc


def _host_prep(x, W, b):
    """bf16 inputs in transposed layouts (see dram tensor comments)."""
    import ml_dtypes
    BF = ml_dtypes.bfloat16
    x = np.asarray(x, np.float32)
    W = np.asarray(W, np.float32)
    b = np.asarray(b, np.float32)
    xtb = x.astype(BF)                       # cast once; transpose per shard
    WTb = np.ascontiguousarray(W.transpose(0, 2, 1)).astype(BF)
    aux = np.stack([W.sum(axis=2), b]).astype(BF)      # [2, L, D] wsum;bias
    return xtb, WTb, aux


def run_shards(x, W, b, **spmd_kwargs):
    """Run the SPMD kernel; returns (full_output, BassKernelResults)."""
    from concourse.bass_utils import run_bass_kernel_spmd

    xtb, WTb, aux = _host_prep(x, W, b)

    if "nc" not in _CACHE:
        _CACHE["nc"] = _build_nc()
    nc = _CACHE["nc"]

    in_maps = []
    for c in range(N_CORES):
        shard = xtb[c * ROWS_PER_CORE:(c + 1) * ROWS_PER_CORE]
        xt = np.ascontiguousarray(shard.T).reshape(NCH, 128, ROWS_PER_CORE)
        in_maps.append({"xt": xt, "wt": WTb, "aux": aux})

    res = run_bass_kernel_spmd(nc, in_maps, core_ids=list(range(N_CORES)),
                               **spmd_kwargs)
    # out[l, c, p, b] -> y[b, l*512 + c*128 + p]
    outs = []
    for r in res.results:
        o = np.asarray(r["out"]).astype(np.float32)
        outs.append(o.transpose(3, 0, 1, 2).reshape(ROWS_PER_CORE,
                                                    NUM_LAYERS * D))
    return np.concatenate(outs, axis=0), res


def kernel(x, W, b):
    out, _ = run_shards(x, W, b)
    return out
